# revision 79
# baseline (speedup 1.0000x reference)
"""Trainium2 Bass kernel for nn_L1RegressionMoEActionHead.

Data-parallel over batch: 16 batch elements -> 2 per core x 8 cores.
Only the selected expert's weights are shipped (host-sliced); scale factors
(1/sqrt(HD), sigmoid(gating)) and LayerNorm gamma/beta are folded into the
weights on the host.  All device GEMMs run in bf16 with fp32 PSUM accumulation.

Layouts on device (per core):
  Q^T/K^T produced transposed ([head_dim, tokens]) with RoPE fused
  (rot via constant pair-swap matmul R, combine on DVE).
  V produced natural ([tokens, dim]).
  Attention in transposed-score space: s^T = K_tile^T . Q^T  -> exp (no
  max-sub; scores are small by construction) -> denom via ones-matmul ->
  O^T = V^T-weighted accumulation.  No transposes needed in attention.
  Then o-proj (+bias via rank-1 matmul, +residual on DVE), LayerNorm
  (bn_stats/bn_aggr), PE-transpose of z, FFN (+bias) + ReLU.
"""

import math
import os

import numpy as np
import ml_dtypes

B = 16
T = 512
KA = 256
KT = 256
DIM = 1024
NH = 8
HD = 128
E = 8
EPS = 1e-5

NCORES = 8
BLOC = B // NCORES          # 2 batch elements per core
TOKQ = BLOC * T             # 1024 query tokens per core
TOKK = BLOC * KA            # 512 kv tokens per core (each of h_a / h_t)
NCT = DIM // 128            # 8 contraction tiles

BF16 = ml_dtypes.bfloat16
F8 = ml_dtypes.float8_e4m3

_CACHE = {}


def _rope_cos_sin(L):
    inv_freq = 1.0 / (10000.0 ** (np.arange(0, HD, 2, dtype=np.float32) / HD))
    freqs = np.arange(L, dtype=np.float32)[:, None] * inv_freq[None, :]
    emb = np.concatenate([freqs, freqs], axis=-1)   # (L, HD)
    return np.cos(emb), np.sin(emb)


def _rhat():
    # rot(q)[2i] = -q[2i+1]; rot(q)[2i+1] = q[2i]  =>  rot = R @ q
    R = np.zeros((HD, HD), dtype=np.float32)
    idx = np.arange(0, HD, 2)
    R[idx, idx + 1] = -1.0
    R[idx + 1, idx] = 1.0
    return R


def build_program():
    import concourse.bass as bass
    import concourse.mybir as mybir
    import concourse.tile as tile
    from concourse import bacc
    from contextlib import ExitStack

    f32 = mybir.dt.float32
    bf16 = mybir.dt.bfloat16
    f8 = mybir.dt.float8e4
    AF = mybir.ActivationFunctionType
    ALU = mybir.AluOpType
    DR = mybir.MatmulPerfMode.DoubleRow

    nc = bacc.Bacc("TRN2", target_bir_lowering=False, debug=False)

    # ---------------- DRAM parameters ----------------
    def din(name, shape, dt):
        return nc.dram_tensor(name, list(shape), dt, kind="ExternalInput")

    xT = din("xT", (DIM, TOKQ), f8)
    xTb = din("xTb", (DIM, TOKQ), bf16)    # bf16 copy for the residual
    haT = din("haT", (DIM, TOKK), f8)
    htT = din("htT", (DIM, TOKK), f8)

    wqaT = din("wqaT", (DIM, DIM), f8)
    wqtT = din("wqtT", (DIM, DIM), f8)
    wkaT = din("wkaT", (DIM, DIM), f8)
    wktT = din("wktT", (DIM, DIM), f8)
    wvaT = din("wvaT", (DIM, DIM), f8)
    wvtT = din("wvtT", (DIM, DIM), f8)
    woT = din("woT", (DIM, DIM), f8)
    wfT = din("wfT", (DIM, DIM), bf16)

    biascols = din("biascols", (128, 5 * NH), f32)   # bqa|bqt|bka|bkt|bo
    bva_b = din("bva_b", (128, DIM), f32)
    bvt_b = din("bvt_b", (128, DIM), f32)
    bfw1 = din("bfw1", (2, DIM), bf16)     # rows: colsums of W'_ffn, b'_ffn

    out_d = nc.dram_tensor("out", [TOKQ, DIM], f32, kind="ExternalOutput")

    # ---------------- inline constants ----------------
    cos_q, sin_q = _rope_cos_sin(T)         # (T, HD)
    cos_k, sin_k = _rope_cos_sin(KA)        # (KA, HD)
    # q tables doubled to TOKQ cols (same positions per batch element) so
    # qa/qt rope runs one 1024-wide pass per head
    cosqT = np.ascontiguousarray(np.tile(cos_q.T, (1, BLOC))).astype(BF16)  # (HD, TOKQ)
    sinqT = np.ascontiguousarray(np.tile(sin_q.T, (1, BLOC))).astype(BF16)
    coskT = np.ascontiguousarray(np.tile(cos_k.T, (1, BLOC))).astype(BF16)  # (HD, TOKK)
    sinkT = np.ascontiguousarray(np.tile(sin_k.T, (1, BLOC))).astype(BF16)

    # pack all bf16 constants into one blob: cols =
    # cosq[0:1024] sinq[1024:2048] cosk[2048:2560] sink[2560:3072]
    # rhatT[3072:3200] ident[3200:3328] ones[3328:3456]
    blob_bf = np.concatenate([
        cosqT, sinqT, coskT, sinkT,
        np.ascontiguousarray(_rhat().T).astype(BF16),
        np.eye(128, dtype=np.float32).astype(BF16),
        np.ones((128, 128), dtype=np.float32).astype(BF16),
    ], axis=1)
    c_blob_bf = nc.inline_tensor(np.ascontiguousarray(blob_bf), "c_blob_bf")
    c_ones8 = nc.inline_tensor(np.ones((128, 32), dtype=F8), "c_ones8")
    # f32 blob: eps[0:1] ones[1:129]
    blob_f = np.concatenate([
        np.full((128, 1), EPS, dtype=np.float32),
        np.ones((128, 128), dtype=np.float32),
    ], axis=1)
    c_blob_f = nc.inline_tensor(np.ascontiguousarray(blob_f), "c_blob_f")

    with tile.TileContext(nc) as tc, ExitStack() as ctx:
        persist = ctx.enter_context(tc.tile_pool(name="persist", bufs=1))
        consts = ctx.enter_context(tc.tile_pool(name="consts", bufs=1))

        # allocate const tiles now; DMAs are issued later, behind the
        # critical-path xT/wqa transfers (the DMA ring is FIFO)
        _cloads = []

        def cload(dram, shape, dt, tag):
            t = consts.tile(list(shape), dt, name=tag, tag=tag)
            _cloads.append((t, dram))
            return t

        sb_cb = cload(c_blob_bf, (128, blob_bf.shape[1]), bf16, "cb")
        sb_cf = cload(c_blob_f, (128, blob_f.shape[1]), f32, "cf")
        sb_bias = cload(biascols, (128, 5 * NH), f32, "biasc")
        sb_cosq = sb_cb[:, 0:1024]
        sb_sinq = sb_cb[:, 1024:2048]
        sb_cosk = sb_cb[:, 2048:2560]
        sb_sink = sb_cb[:, 2560:3072]
        sb_rhatT = sb_cb[:, 3072:3200]
        sb_ident = sb_cb[:, 3200:3328]
        sb_ones_col = sb_cb[:, 3328:3329]
        sb_ones_row = sb_cb[0:1, 3328:3456]
        sb_ones_row_f = sb_cf[0:1, 1:129]
        sb_eps = sb_cf[:, 0:1]
        sb_bqa = sb_bias[:, 0:NH]
        sb_bqt = sb_bias[:, NH:2 * NH]
        sb_bka = sb_bias[:, 2 * NH:3 * NH]
        sb_bkt = sb_bias[:, 3 * NH:4 * NH]
        sb_bo_col = sb_bias[:, 4 * NH:5 * NH]
        sb_w1b = cload(bfw1, (2, DIM), bf16, "bfw1")
        sb_ones8 = consts.tile([128, 2, 16], f8, name="ones8", tag="ones8")

        # persistent activation tiles
        qa_sb = [persist.tile([HD, TOKQ], bf16, name=f"qa{h}", tag=f"qa{h}") for h in range(NH)]
        qt_sb = [persist.tile([HD, TOKQ], bf16, name=f"qt{h}", tag=f"qt{h}") for h in range(NH)]
        ka_sb = [persist.tile([HD, TOKK], bf16, name=f"ka{h}", tag=f"ka{h}") for h in range(NH)]
        kt_sb = [persist.tile([HD, TOKK], bf16, name=f"kt{h}", tag=f"kt{h}") for h in range(NH)]
        # v stored fp8, key-blocks paired in dim1 for DoubleRow attention
        va_sb = [persist.tile([128, 2, DIM], f8, name=f"va{b}", tag=f"va{b}") for b in range(BLOC)]
        vt_sb = [persist.tile([128, 2, DIM], f8, name=f"vt{b}", tag=f"vt{b}") for b in range(BLOC)]
        # attention output fp8, heads paired in dim1 for the DoubleRow o-proj
        o_sb = [persist.tile([128, NH, T], f8, name=f"o{b}", tag=f"o{b}")
                for b in range(BLOC)]

        # w2 pool created early so wo/wf prefetch overlaps Phases A/B
        w2 = ctx.enter_context(tc.tile_pool(name="w2", bufs=1))

        # ================= Phase A: projections (fp8 DoubleRow) ==========
        # All six input projections run in fp8 e4m3 with DoubleRow matmuls:
        # one MM contracts a pair of 128-row chunks (256 of the 1024-dim
        # contraction), so each 512-token output needs 4 MMs instead of 8.
        with tc.tile_pool(name="acts", bufs=1) as actp, \
             tc.tile_pool(name="wpool", bufs=3) as wpool, \
             tc.tile_pool(name="ptmp", bufs=6) as ptmp, \
             tc.tile_pool(name="ppsum", bufs=2, space="PSUM") as ppsum, \
             tc.tile_pool(name="rpsum", bufs=3, space="PSUM") as rpsum:

            # critical-path DMAs first (DMA ring drains in issue order):
            # x halves interleaved with wqa halves and the small consts
            # split the critical startup DMAs over two engine queues so the
            # transfers overlap (a dma_start blocks its issuing engine)
            sb_xT = actp.tile([128, NCT, TOKQ], f8, tag="xT")
            x_src = xT.ap().rearrange("(a p) t -> p a t", p=128)
            w_qa_t = wpool.tile([128, NCT, DIM], f8, name="w", tag="w")
            wqa_src = wqaT.ap().rearrange("(a p) j -> p a j", p=128)
            nc.sync.dma_start(sb_xT[:, 0:4, :], x_src[:, 0:4, :])
            nc.scalar.dma_start(w_qa_t[:, :, 0:512], wqa_src[:, :, 0:512])
            nc.sync.dma_start(sb_xT[:, 4:8, :], x_src[:, 4:8, :])
            nc.scalar.dma_start(w_qa_t[:, :, 512:1024], wqa_src[:, :, 512:1024])
            # warm up the PE clock (HAM) from memset tiles — no DMA dep
            wsL = ptmp.tile([128, 128], bf16, tag="wsL", bufs=1)
            nc.vector.memset(wsL[:], 0.0)
            wsR = ptmp.tile([128, 512], bf16, tag="wsR", bufs=1)
            nc.vector.memset(wsR[:], 0.0)
            wsink = rpsum.tile([128, 512], f32, tag="rot")
            for _ in range(16):
                nc.tensor.matmul(wsink[:], wsL[:], wsR[:],
                                 start=True, stop=True)

            nc.scalar.dma_start(_cloads[0][0][:], _cloads[0][1].ap())   # cb
            nc.scalar.dma_start(_cloads[2][0][:], _cloads[2][1].ap())   # biascols
            for ci in (1, 3):      # cf, bfw1
                nc.scalar.dma_start(_cloads[ci][0][:], _cloads[ci][1].ap())
            nc.scalar.dma_start(
                sb_ones8[:],
                c_ones8.ap().rearrange("p (a b) -> p a b", b=16))

            def load_w(wdram):
                t = wpool.tile([128, NCT, DIM], f8, name="w", tag="w")
                nc.sync.dma_start(
                    t[:], wdram.ap().rearrange("(a p) j -> p a j", p=128))
                return t

            def qk_stage(wdram, bias_sb, src_sb, tok_len, out_tiles, costab,
                         sintab, w=None):
                if w is None:
                    w = load_w(wdram)
                for j in range(NH):
                    jsl = slice(j * 128, (j + 1) * 128)
                    ps = ppsum.tile([128, tok_len], f32, tag="proj",
                                    padded_shape=[128, TOKQ])
                    for half in range(tok_len // 512):
                        hsl = slice(half * 512, (half + 1) * 512)
                        for c4 in range(4):
                            nc.tensor.matmul(
                                ps[:, hsl], w[:, 2 * c4:2 * c4 + 2, jsl],
                                src_sb[:, 2 * c4:2 * c4 + 2, hsl],
                                start=(c4 == 0), stop=(c4 == 3), perf_mode=DR)
                    q1 = ptmp.tile([128, tok_len], bf16, tag="q1",
                                   padded_shape=[128, TOKQ])
                    nc.scalar.activation(q1[:], ps[:], AF.Identity,
                                         bias=bias_sb[:, j:j + 1])
                    rotb = ptmp.tile([128, tok_len], bf16, tag="rotb",
                                     padded_shape=[128, TOKQ])
                    for half in range(tok_len // 512):
                        hsl = slice(half * 512, (half + 1) * 512)
                        rot = rpsum.tile([128, 512], f32, tag="rot")
                        nc.tensor.matmul(rot[:], sb_rhatT[:], q1[:, hsl],
                                         start=True, stop=True)
                        nc.scalar.copy(rotb[:, hsl], rot[:])
                    t1 = ptmp.tile([128, tok_len], bf16, tag="t1",
                                   padded_shape=[128, TOKQ])
                    nc.vector.tensor_tensor(t1[:], q1[:], costab[:, 0:tok_len],
                                            op=ALU.mult)
                    t2 = ptmp.tile([128, tok_len], bf16, tag="t2",
                                   padded_shape=[128, TOKQ])
                    nc.vector.tensor_tensor(t2[:], rotb[:], sintab[:, 0:tok_len],
                                            op=ALU.mult)
                    nc.vector.tensor_tensor(out_tiles[j][:], t1[:], t2[:],
                                            op=ALU.add)

            def v_stage(wdram, src_sb, out_tiles, bias_bcast):
                w = load_w(wdram)
                for kt_i in range(TOKK // 128):
                    ksl = slice(kt_i * 128, (kt_i + 1) * 128)
                    ps = ppsum.tile([128, DIM], f32, tag="proj")
                    for jc in range(2):
                        sl = slice(jc * 512, (jc + 1) * 512)
                        for c4 in range(4):
                            nc.tensor.matmul(
                                ps[:, sl], src_sb[:, 2 * c4:2 * c4 + 2, ksl],
                                w[:, 2 * c4:2 * c4 + 2, sl],
                                start=(c4 == 0), stop=(c4 == 3), perf_mode=DR)
                    nc.vector.tensor_tensor(out_tiles[kt_i // 2][:, kt_i % 2, :],
                                            ps[:], bias_bcast[:], op=ALU.add)

            qk_stage(wqaT, sb_bqa, sb_xT, TOKQ, qa_sb, sb_cosq, sb_sinq,
                     w=w_qa_t)
            # remaining input DMAs issue here, behind the critical-path ones
            sb_haT = actp.tile([128, NCT, TOKK], f8, tag="haT")
            nc.sync.dma_start(sb_haT[:], haT.ap().rearrange("(a p) t -> p a t", p=128))
            sb_htT = actp.tile([128, NCT, TOKK], f8, tag="htT")
            nc.sync.dma_start(sb_htT[:], htT.ap().rearrange("(a p) t -> p a t", p=128))
            sb_bva = actp.tile([128, DIM], f32, name="bva", tag="bva")
            nc.sync.dma_start(sb_bva[:], bva_b.ap())
            sb_bvt = actp.tile([128, DIM], f32, name="bvt", tag="bvt")
            nc.sync.dma_start(sb_bvt[:], bvt_b.ap())

            qk_stage(wqtT, sb_bqt, sb_xT, TOKQ, qt_sb, sb_cosq, sb_sinq)
            qk_stage(wkaT, sb_bka, sb_haT, TOKK, ka_sb, sb_cosk, sb_sink)
            v_stage(wvaT, sb_haT, va_sb, sb_bva)
            qk_stage(wktT, sb_bkt, sb_htT, TOKK, kt_sb, sb_cosk, sb_sink)
            v_stage(wvtT, sb_htT, vt_sb, sb_bvt)

            # prefetch Phase C weights; lands well before attention finishes
            wo8 = w2.tile([128, NCT, DIM], f8, name="wo8", tag="wo")
            nc.sync.dma_start(wo8[:], woT.ap().rearrange("(a p) j -> p a j", p=128))
            wft = w2.tile([128, NCT, DIM], bf16, name="wft", tag="wf")
            nc.sync.dma_start(wft[:], wfT.ap().rearrange("(a p) j -> p a j", p=128))

        # bf16 x^T for the residual, loaded once; DMA overlaps Phase B
        xtb_pool = ctx.enter_context(tc.tile_pool(name="xtbp", bufs=1))
        xtb = xtb_pool.tile([128, NCT, TOKQ], bf16, tag="xtb")
        nc.sync.dma_start(xtb[:], xTb.ap().rearrange("(a p) t -> p a t", p=128))

        # ================= Phase B: attention (fp8 p/v DoubleRow) ========
        # scores per group (audio/text) land in one 2-bank PSUM tile, one
        # exp covers both key blocks, and ov/den contract 256 keys per
        # DoubleRow matmul with fp8 p and v.
        with tc.tile_pool(name="atmp", bufs=6) as atmp, \
             tc.tile_pool(name="artmp", bufs=3) as artmp, \
             tc.tile_pool(name="aps", bufs=2, space="PSUM") as aps:
            ones8_dr = sb_ones8[:, :, 0:1]
            for b in range(BLOC):
                for h in range(NH):
                    den = aps.tile([1, 512], f32, tag="den", bufs=1)
                    ov = aps.tile([128, 512], f32, tag="ov")
                    qsl = slice(b * T, (b + 1) * T)
                    for grp in range(2):
                        if grp == 0:
                            ksb, qsb, v8 = ka_sb[h], qa_sb[h], va_sb[b]
                        else:
                            ksb, qsb, v8 = kt_sb[h], qt_sb[h], vt_sb[b]
                        s = aps.tile([128, 1024], f32, tag="s")
                        for blk in range(2):
                            koff = b * KA + blk * 128
                            nc.tensor.matmul(s[:, blk * 512:(blk + 1) * 512],
                                             ksb[:, koff:koff + 128],
                                             qsb[:, qsl], start=True, stop=True)
                        p8 = atmp.tile([128, 1024], f8, tag="p")
                        nc.scalar.activation(p8[:], s[:], AF.Exp)
                        p3 = p8.rearrange("p (a b) -> p a b", a=2)
                        nc.tensor.matmul(den[:], ones8_dr, p3,
                                         start=(grp == 0), stop=(grp == 1),
                                         perf_mode=DR, skip_group_check=True)
                        nc.tensor.matmul(ov[:], v8[:, :, h * 128:(h + 1) * 128],
                                         p3, start=(grp == 0), stop=(grp == 1),
                                         perf_mode=DR, skip_group_check=True)
                    recip = artmp.tile([1, 512], f32, tag="recip")
                    nc.vector.reciprocal_approx_fast(recip[:], den[:])
                    recip_bf = artmp.tile([1, 512], bf16, tag="recip_bf")
                    nc.vector.tensor_copy(recip_bf[:], recip[:])
                    rbps = aps.tile([128, 512], f32, tag="rbps", bufs=1)
                    nc.tensor.matmul(rbps[:], sb_ones_row[:], recip_bf[:],
                                     start=True, stop=True)
                    rb = artmp.tile([128, 512], f32, tag="rb")
                    nc.vector.tensor_copy(rb[:], rbps[:])
                    nc.vector.tensor_tensor(o_sb[b][:, h, :], ov[:], rb[:],
                                            op=ALU.mult)

        # ========== Phase C: o-projT (fp8 DR) + MM-LayerNorm + FFN =======
        # o-proj runs transposed (x2^T[dout, t]) so b_o is a per-partition
        # ACT bias and the residual comes from the resident x^T. LayerNorm
        # stats are ones-matmuls over partitions; z stays transposed, which
        # lets the FFN produce the final output in natural layout with no
        # transposes at all.
        # The LayerNorm application is folded into the FFN: the FFN runs on
        # raw x2 and the identity
        #   relu(rstd·(x2·W' − mu⊗w1 + std⊗b'))
        #   (w1 = colsums of W', b' = effective bias, rstd via ACT scale)
        # makes the normalization a rank-2 matmul + a per-partition scale,
        # so nothing on the FFN path waits for a z tensor.
        with tc.tile_pool(name="ctmp", bufs=3) as ctmp, \
             tc.tile_pool(name="cz", bufs=1) as czp, \
             tc.tile_pool(name="cres", bufs=3) as cres, \
             tc.tile_pool(name="cps", bufs=2, space="PSUM") as cps:

            wf = [wft[:, ct, :] for ct in range(NCT)]

            def c_proj(b):
                x2tb = czp.tile([128, NCT, T], bf16, name=f"x2_{b}",
                                tag=f"x2_{b}")
                sqs = czp.tile([128, NCT, T], bf16, name=f"sq_{b}",
                               tag=f"sq_{b}")
                for dc in range(NCT):
                    dsl = slice(dc * 128, (dc + 1) * 128)
                    ps = cps.tile([128, T], f32, tag="op")
                    for c in range(4):
                        nc.tensor.matmul(
                            ps[:], wo8[:, 2 * c:2 * c + 2, dsl],
                            o_sb[b][:, 2 * c:2 * c + 2, :],
                            start=(c == 0), stop=(c == 3), perf_mode=DR)
                    ob = ctmp.tile([128, T], bf16, tag="ob")
                    nc.scalar.activation(ob[:], ps[:], AF.Identity,
                                         bias=sb_bo_col[:, dc:dc + 1])
                    nc.vector.tensor_tensor(x2tb[:, dc, :], ob[:],
                                            xtb[:, dc, b * T:(b + 1) * T],
                                            op=ALU.add)
                    nc.vector.tensor_tensor(sqs[:, dc, :], x2tb[:, dc, :],
                                            x2tb[:, dc, :], op=ALU.mult)
                return x2tb, sqs

            def c_stats(b, x2tb, sqs):
                mu_ps = cps.tile([1, T], f32, tag="mu")
                ss_ps = cps.tile([1, T], f32, tag="ss")
                for dc in range(NCT):
                    nc.tensor.matmul(mu_ps[:], sb_ones_col[:], x2tb[:, dc, :],
                                     start=(dc == 0), stop=(dc == NCT - 1),
                                     skip_group_check=True)
                    nc.tensor.matmul(ss_ps[:], sb_ones_col[:], sqs[:, dc, :],
                                     start=(dc == 0), stop=(dc == NCT - 1),
                                     skip_group_check=True)
                return mu_ps, ss_ps

            def c_chain(b, mu_ps, ss_ps):
                mrow = ctmp.tile([1, T], f32, tag="mrow")
                nc.vector.tensor_scalar(mrow[:], mu_ps[:], scalar1=1.0 / DIM,
                                        scalar2=None, op0=ALU.mult)
                vrow = ctmp.tile([1, T], f32, tag="vrow")
                nc.vector.tensor_scalar(vrow[:], ss_ps[:], scalar1=1.0 / DIM,
                                        scalar2=None, op0=ALU.mult)
                msq = ctmp.tile([1, T], f32, tag="msq")
                nc.vector.tensor_tensor(msq[:], mrow[:], mrow[:], op=ALU.mult)
                nc.vector.tensor_tensor(vrow[:], vrow[:], msq[:],
                                        op=ALU.subtract)
                sd = ctmp.tile([1, T], f32, tag="sd")
                nc.scalar.activation(sd[:], vrow[:], AF.Sqrt,
                                     bias=sb_eps[0:1, 0:1])
                musd = ctmp.tile([2, T], f32, tag="musd")
                nc.vector.tensor_scalar(musd[0:1, :], mrow[:], scalar1=-1.0,
                                        scalar2=None, op0=ALU.mult)
                nc.sync.dma_start(musd[1:2, :], sd[:])
                musd_bf = ctmp.tile([2, T], bf16, tag="musdbf")
                nc.vector.tensor_copy(musd_bf[:], musd[:])
                rs = ctmp.tile([1, T], f32, tag="rs")
                nc.vector.reciprocal(rs[:], sd[:])
                # rstd as per-partition columns for the ACT relu scale
                rstdc = ctmp.tile([128, T // 128], f32, tag="rstdc")
                for t4 in range(T // 128):
                    tp = cps.tile([128, 1], f32, tag="op")
                    nc.tensor.matmul(tp[:], rs[0:1, t4 * 128:(t4 + 1) * 128],
                                     sb_ones_row_f[0:1, 0:1],
                                     start=True, stop=True)
                    nc.vector.tensor_copy(rstdc[:, t4:t4 + 1], tp[:])
                return musd_bf, rstdc

            def c_ffn(b, x2tb, musd_bf, rstdc):
                for t4 in range(T // 128):
                    tq = slice(t4 * 128, (t4 + 1) * 128)
                    row0 = b * T + t4 * 128
                    for jc in range(2):
                        sl = slice(jc * 512, (jc + 1) * 512)
                        fp = cps.tile([128, 512], f32, tag="fp")
                        for ct in range(NCT):
                            nc.tensor.matmul(fp[:], x2tb[:, ct, tq],
                                             wf[ct][:, sl],
                                             start=(ct == 0), stop=False)
                        nc.tensor.matmul(fp[:], musd_bf[:, tq],
                                         sb_w1b[:, sl],
                                         start=False, stop=True)
                        res = cres.tile([128, 512], f32, tag="res")
                        nc.scalar.activation(res[:], fp[:], AF.Relu,
                                             scale=rstdc[:, t4:t4 + 1])
                        nc.sync.dma_start(out_d.ap()[row0:row0 + 128, sl],
                                          res[:])

            st = [c_proj(b) for b in range(BLOC)]
            stats = [c_stats(b, *st[b]) for b in range(BLOC)]
            chains = [c_chain(b, *stats[b]) for b in range(BLOC)]
            for b in range(BLOC):
                c_ffn(b, st[b][0], *chains[b])

    nc.compile()
    return nc


def _prep_host(inputs):
    """Host-side preprocessing: expert select, folding, transposes, sharding."""
    x = np.asarray(inputs["x"], dtype=np.float32)
    h_a = np.asarray(inputs["h_a"], dtype=np.float32)
    h_t = np.asarray(inputs["h_t"], dtype=np.float32)
    e = int(np.asarray(inputs["expert_idx"]))
    g = float(1.0 / (1.0 + math.exp(-float(np.asarray(inputs["gating_factor"])[e]))))
    sc = 1.0 / math.sqrt(HD)

    def wT(w, scale=1.0):
        return np.ascontiguousarray(
            (np.asarray(w, dtype=np.float32)[e] * scale).T).astype(BF16)

    def wT8(w, scale=1.0):
        return np.ascontiguousarray(
            (np.asarray(w, dtype=np.float32)[e] * scale).T).astype(F8)

    def brow(bv, scale=1.0, dtype=BF16):
        return (np.asarray(bv, dtype=np.float32)[e] * scale).reshape(1, DIM).astype(dtype)

    def bcol(bv, scale=1.0):
        # [DIM] -> [128, NH]: column h = b[h*128:(h+1)*128]
        return np.ascontiguousarray(
            (np.asarray(bv, dtype=np.float32)[e] * scale).reshape(NH, 128).T
        ).astype(np.float32)

    gamma = np.asarray(inputs["gamma"], dtype=np.float32)[e]
    beta = np.asarray(inputs["beta"], dtype=np.float32)[e]
    w_ffn = np.asarray(inputs["W_ffn"], dtype=np.float32)[e]
    b_ffn = np.asarray(inputs["b_ffn"], dtype=np.float32)[e]
    w_f_eff = w_ffn * gamma[None, :]
    b_f_eff = b_ffn + w_ffn @ beta

    shared = {
        "wqaT": wT8(inputs["W_qa"], sc),
        "wqtT": wT8(inputs["W_qt"], sc * g),
        "wkaT": wT8(inputs["W_ka"]),
        "wktT": wT8(inputs["W_kt"]),
        "wvaT": wT8(inputs["W_va"]),
        "wvtT": wT8(inputs["W_vt"]),
        "woT": wT8(inputs["W_o"]),
        "wfT": np.ascontiguousarray(w_f_eff.T).astype(BF16),
        "biascols": np.ascontiguousarray(np.concatenate([
            bcol(inputs["b_qa"], sc),
            bcol(inputs["b_qt"], sc * g),
            bcol(inputs["b_ka"]),
            bcol(inputs["b_kt"]),
            bcol(inputs["b_o"]),
        ], axis=1)),
        "bva_b": np.ascontiguousarray(np.tile(
            np.asarray(inputs["b_va"], dtype=np.float32)[e][None, :], (128, 1))),
        "bvt_b": np.ascontiguousarray(np.tile(
            np.asarray(inputs["b_vt"], dtype=np.float32)[e][None, :], (128, 1))),
        "bfw1": np.ascontiguousarray(np.stack(
            [w_f_eff.sum(axis=1), b_f_eff])).astype(BF16),
    }

    in_maps = []
    for c in range(NCORES):
        xc = x[c * BLOC:(c + 1) * BLOC].reshape(TOKQ, DIM)
        hac = h_a[c * BLOC:(c + 1) * BLOC].reshape(TOKK, DIM)
        htc = h_t[c * BLOC:(c + 1) * BLOC].reshape(TOKK, DIM)
        m = dict(shared)
        xcT = np.ascontiguousarray(xc.T)
        m["xT"] = xcT.astype(F8)
        m["xTb"] = xcT.astype(BF16)
        m["haT"] = np.ascontiguousarray(hac.T).astype(F8)
        m["htT"] = np.ascontiguousarray(htc.T).astype(F8)
        in_maps.append(m)
    return in_maps


def run(inputs, trace=False):
    from concourse.bass_utils import run_bass_kernel_spmd

    if "nc" not in _CACHE:
        _CACHE["nc"] = build_program()
    nc = _CACHE["nc"]
    in_maps = _prep_host(inputs)
    res = run_bass_kernel_spmd(nc, in_maps, list(range(NCORES)), trace=trace)
    outs = [res.results[c]["out"].reshape(BLOC, T, DIM) for c in range(NCORES)]
    return np.concatenate(outs, axis=0), res


def kernel(**inputs) -> np.ndarray:
    out, _ = run(inputs, trace=False)
    return out

